# revision 1
# baseline (speedup 1.0000x reference)
"""Linearized attention Trainium2 kernel.

Reference computation per batch b (C=64 channels, H=W=256, N=65536 pixels,
2 heads x 32 head-dim):
    qkv   = qkv_w @ x                      # per-pixel 1x1 conv
    q,k,v = split(qkv); phi(t) = elu(t)+1
    KV    = phi(k) @ v.T  (per head, contract over pixels)   # [32, 32]
    out_h = KV.T @ phi(q) (per head)
    y     = proj_w @ out_h

Sharding: data-parallel over batch, 1 batch per NeuronCore (8 cores).

Kernel structure per core:
- x is fed as bf16 (host cast; device matmuls are bf16 anyway).
- The 65536-pixel image is two half-images ("A" = pixels 0:32768,
  "B" = 32768:65536). SBUF tiles pack A on partitions 0:64, B on 64:128
  where possible, but every matmul keeps its contraction on partitions
  0:64 (mixing disjoint K row-groups between consecutive 128-column
  bf16-weight matmuls crashes the PE array - FWL row-group hazard).
- phi is exact:  phi(x) = max(x, 0) + min(exp(x), 1).
- Pass 1, per [*, 512]-column tile: load x -> one PSUM tile holds
  q (c-major, cols 0:512), kT (pixel-major, cols 512:1024) and vT
  (cols 1024:1536); kT/vT come from matmuls with the x-chunk as the
  *stationary* operand writing a strided 3-D out AP (no transpose
  instructions). One ACT exp covers q+kT; one DVE 4x min clamps; two
  fused (max 0)+add ops produce the phi(q) stash slice and phi(k)T.
  KV^T accumulates in PSUM over all 512 pixel-chunks.
- Boundary: W2 = blockdiag(KV) @ proj_w.T folded into a single [64,64]
  weight, so pass 2 is just  y = W2.T @ phi(q)  per tile + store.
"""

import sys

if "/opt/trn_rl_repo" not in sys.path:
    sys.path.insert(0, "/opt/trn_rl_repo")

import numpy as np
import ml_dtypes

import concourse.bacc as bacc
import concourse.bass as bass
import concourse.mybir as mybir
import concourse.tile as tile
from concourse.bass_utils import run_bass_kernel_spmd

AF = mybir.ActivationFunctionType
ALU = mybir.AluOpType
F32 = mybir.dt.float32
BF16 = mybir.dt.bfloat16

B, C, H, W = 8, 64, 256, 256
N = H * W            # pixels per batch
HALF = N // 2        # pixels per half-image
NT = 512             # tile width (columns per half-image per tile)
NTILES = HALF // NT  # 64
CHUNKS = 2 * NT // 128  # transposed 128-pixel chunks per tile: 8

_cached = None


def _build():
    nc = bacc.Bacc("TRN2", target_bir_lowering=False, debug=False)

    x_d = nc.dram_tensor("x", [C, N], BF16, kind="ExternalInput")
    wq_d = nc.dram_tensor("wq", [64, 64], BF16, kind="ExternalInput")
    wkv_d = nc.dram_tensor("wkv", [64, 128], BF16, kind="ExternalInput")
    pj_d = nc.dram_tensor("pj", [64, 64], BF16, kind="ExternalInput")
    y_d = nc.dram_tensor("y", [C, N], F32, kind="ExternalOutput")

    x_ap = x_d.ap()
    y_ap = y_d.ap()

    with tile.TileContext(nc) as tc:
        with (
            tc.tile_pool(name="persist", bufs=1) as persist,
            tc.tile_pool(name="stash", bufs=1) as stash_pool,
        ):
            wq = persist.tile([64, 64], BF16)
            wkv = persist.tile([64, 128], BF16)
            pj = persist.tile([64, 64], BF16)
            w2 = persist.tile([128, 64], BF16)
            kvbd = persist.tile([64, 64], BF16)
            nc.sync.dma_start(wq[:], wq_d.ap())
            nc.sync.dma_start(wkv[:], wkv_d.ap())
            nc.sync.dma_start(pj[:], pj_d.ap())
            nc.gpsimd.memset(kvbd[:], 0.0)

            # phi(q) stash: c-major, half A rows 0:64, half B rows 64:128
            stash = stash_pool.tile([128, HALF], BF16)

            # ---------------- pass 1 ----------------
            with (
                tc.tile_pool(name="xb", bufs=4) as xb_pool,
                tc.tile_pool(name="p1sb", bufs=4) as p1sb,
                tc.tile_pool(name="qps", bufs=2, space="PSUM") as qps_pool,
                tc.tile_pool(name="kvtps", bufs=2, space="PSUM") as kvt_pool,
                tc.tile_pool(name="kvacc", bufs=1, space="PSUM") as kvacc_pool,
            ):
                kvacc = kvacc_pool.tile([64, 64], F32, tag="kvacc")

                for t in range(NTILES):
                    cs = bass.ts(t, NT)  # column slice within each half

                    # de-packed bf16 x tile: cols 0:512 half A, 512:1024 half B
                    xb = xb_pool.tile([64, 2 * NT], BF16)
                    nc.sync.dma_start(
                        xb[:],
                        bass.AP(x_d, t * NT, [[N, 64], [HALF, 2], [1, NT]]),
                    )

                    # q (c-major): halves packed on partitions
                    q_ps = qps_pool.tile([128, NT], F32)
                    nc.tensor.matmul(
                        q_ps[0:64, :], wq[:], xb[:, 0:NT],
                        start=True, stop=True,
                    )
                    nc.tensor.matmul(
                        q_ps[64:128, :], wq[:], xb[:, NT:2 * NT],
                        start=True, stop=True, tile_position=(0, 64),
                    )
                    # kT/vT chunks: x-chunk stationary; chunk s cols
                    # [s*128, s*128+64) = kT, [+64, +128) = vT
                    kvt = kvt_pool.tile([128, CHUNKS * 128], F32)
                    for s in range(CHUNKS):
                        nc.tensor.matmul(
                            kvt[:, bass.ts(s, 128)],
                            xb[:, bass.ts(s, 128)],
                            wkv[:],
                            start=True, stop=True,
                        )
                    kvt3 = kvt[:].rearrange("p (s c) -> p s c", s=CHUNKS)

                    # phi = max(x,0) + min(exp(x),1) for q and kT
                    eqk = p1sb.tile([128, 2 * NT], BF16, tag="eqk")
                    nc.scalar.activation(eqk[:, 0:NT], q_ps[:], AF.Exp)
                    nc.scalar.activation(
                        eqk[:, NT:2 * NT].rearrange("p (s c) -> p s c", s=CHUNKS),
                        kvt3[:, :, 0:64], AF.Exp,
                    )
                    mine = p1sb.tile([128, 2 * NT], BF16, tag="mine")
                    nc.vector.tensor_scalar_min(mine[:], eqk[:], 1.0)
                    nc.vector.scalar_tensor_tensor(
                        stash[:, cs], q_ps[:], 0.0, mine[:, 0:NT],
                        op0=ALU.max, op1=ALU.add,
                    )
                    kphiT = p1sb.tile([128, NT], BF16, tag="kphiT")
                    nc.vector.scalar_tensor_tensor(
                        kphiT[:].rearrange("p (s c) -> p s c", s=CHUNKS),
                        kvt3[:, :, 0:64], 0.0,
                        mine[:, NT:2 * NT].rearrange("p (s c) -> p s c", s=CHUNKS),
                        op0=ALU.max, op1=ALU.add,
                    )
                    # vT -> SBUF bf16; split between ACT and DVE for balance
                    vt = p1sb.tile([128, NT], BF16, tag="vt")
                    nc.scalar.copy(
                        vt[:, 0:NT // 2].rearrange("p (s c) -> p s c", s=CHUNKS // 2),
                        kvt3[:, 0:CHUNKS // 2, 64:128],
                    )
                    nc.vector.tensor_copy(
                        vt[:, NT // 2:NT].rearrange("p (s c) -> p s c", s=CHUNKS // 2),
                        kvt3[:, CHUNKS // 2:CHUNKS, 64:128],
                    )

                    # KV^T accumulation (both heads; off-diag ignored later)
                    for s in range(CHUNKS):
                        nc.tensor.matmul(
                            kvacc[:],
                            vt[:, bass.ts(s, 64)],
                            kphiT[:, bass.ts(s, 64)],
                            start=(t == 0 and s == 0),
                            stop=(t == NTILES - 1 and s == CHUNKS - 1),
                            skip_group_check=True,
                        )

                # block-diagonal KV^T (cross-head garbage dropped)
                for h0, h1 in ((0, 32), (32, 64)):
                    nc.vector.tensor_copy(
                        kvbd[h0:h1, h0:h1], kvacc[h0:h1, h0:h1]
                    )

            # ---------------- boundary: W2 = blockdiag(KV) @ proj.T ------
            with tc.tile_pool(name="bps", bufs=1, space="PSUM") as bps:
                w2ps = bps.tile([64, 64], F32)
                nc.tensor.matmul(w2ps[:], kvbd[:], pj[:], start=True, stop=True)
                nc.vector.tensor_copy(w2[0:64, :], w2ps[:])
                nc.vector.tensor_copy(w2[64:128, :], w2ps[:])

            # ---------------- pass 2: y = W2.T @ phi(q) ----------------
            with (
                tc.tile_pool(name="p2sb", bufs=4) as p2sb,
                tc.tile_pool(name="yps", bufs=3, space="PSUM") as yps_pool,
            ):
                for t in range(NTILES):
                    cs = bass.ts(t, NT)
                    y_ps = yps_pool.tile([128, NT], F32)
                    nc.tensor.matmul(
                        y_ps[0:64, :], w2[0:64, :], stash[0:64, cs],
                        start=True, stop=True,
                    )
                    nc.tensor.matmul(
                        y_ps[64:128, :], w2[64:128, :], stash[64:128, cs],
                        start=True, stop=True, tile_position=(64, 64),
                    )
                    ysb = p2sb.tile([128, NT], F32, tag="y")
                    nc.scalar.copy(ysb[:], y_ps[:])
                    nc.sync.dma_start(
                        bass.AP(y_d, t * NT, [[HALF, 2], [N, 64], [1, NT]]),
                        ysb[:],
                    )

    nc.compile()
    return nc


def _get_nc():
    global _cached
    if _cached is None:
        _cached = _build()
    return _cached


def _prep_weights(qkv_w, proj_w):
    wq = np.ascontiguousarray(qkv_w[0:64].T).astype(ml_dtypes.bfloat16)
    wkT = qkv_w[64:128].T
    wvT = qkv_w[128:192].T
    wkv = np.ascontiguousarray(
        np.concatenate([wkT, wvT], axis=1)
    ).astype(ml_dtypes.bfloat16)
    pj = np.ascontiguousarray(proj_w.T).astype(ml_dtypes.bfloat16)
    return wq, wkv, pj


def run(x, qkv_w, proj_w, trace=False):
    nc = _get_nc()
    wq, wkv, pj = _prep_weights(np.asarray(qkv_w), np.asarray(proj_w))
    x = np.asarray(x)
    in_maps = [
        {
            "x": np.ascontiguousarray(x[b].reshape(C, N)).astype(
                ml_dtypes.bfloat16
            ),
            "wq": wq,
            "wkv": wkv,
            "pj": pj,
        }
        for b in range(B)
    ]
    res = run_bass_kernel_spmd(nc, in_maps, core_ids=list(range(B)), trace=trace)
    out = np.stack([res.results[b]["y"].reshape(C, H, W) for b in range(B)])
    return out.astype(np.float32), res


def kernel(x, qkv_w, proj_w):
    out, _ = run(x, qkv_w, proj_w, trace=False)
    return out



# revision 6
# speedup vs baseline: 1.4153x; 1.4153x over previous
"""Linearized attention Trainium2 kernel.

Reference computation per batch b (C=64 channels, H=W=256, N=65536 pixels,
2 heads x 32 head-dim):
    qkv   = qkv_w @ x                      # per-pixel 1x1 conv
    q,k,v = split(qkv); phi(t) = elu(t)+1
    KV    = phi(k) @ v.T  (per head, contract over pixels)   # [32, 32]
    out_h = KV.T @ phi(q) (per head)
    y     = proj_w @ out_h

Sharding: data-parallel over batch, 1 batch per NeuronCore (8 cores).

Kernel structure per core:
- x is fed as bf16 (host cast; device matmuls are bf16 anyway).
- The 65536-pixel image is two half-images ("A" = pixels 0:32768,
  "B" = 32768:65536). SBUF tiles pack A on partitions 0:64, B on 64:128
  where possible, but every matmul keeps its contraction on partitions
  0:64 (mixing disjoint K row-groups between consecutive 128-column
  bf16-weight matmuls crashes the PE array - FWL row-group hazard).
- phi is exact:  phi(x) = max(x, 0) + min(exp(x), 1).
- Pass 1, per [*, 512]-column tile: load x -> one PSUM tile holds
  q (c-major, cols 0:512), kT (pixel-major, cols 512:1024) and vT
  (cols 1024:1536); kT/vT come from matmuls with the x-chunk as the
  *stationary* operand writing a strided 3-D out AP (no transpose
  instructions). One ACT exp covers q+kT; one DVE 4x min clamps; two
  fused (max 0)+add ops produce the phi(q) stash slice and phi(k)T.
  KV^T accumulates in PSUM over all 512 pixel-chunks.
- Boundary: W2 = blockdiag(KV) @ proj_w.T folded into a single [64,64]
  weight, so pass 2 is just  y = W2.T @ phi(q)  per tile + store.
"""

import sys

if "/opt/trn_rl_repo" not in sys.path:
    sys.path.insert(0, "/opt/trn_rl_repo")

import numpy as np
import ml_dtypes

import concourse.bacc as bacc
import concourse.bass as bass
import concourse.mybir as mybir
import concourse.tile as tile
from concourse.bass_utils import run_bass_kernel_spmd

AF = mybir.ActivationFunctionType
ALU = mybir.AluOpType
F32 = mybir.dt.float32
BF16 = mybir.dt.bfloat16

B, C, H, W = 8, 64, 256, 256
N = H * W            # pixels per batch
HALF = N // 2        # pixels per half-image
NT = 512             # tile width (columns per half-image per tile)
NTILES = HALF // NT  # 64
CHUNKS = 2 * NT // 128  # transposed 128-pixel chunks per tile: 8
CHUNK_PX = 4096      # pixels per half per input-load chunk
NCHUNKS = HALF // CHUNK_PX  # 8 input DMAs
TPC = CHUNK_PX // NT        # tiles per load chunk: 8
YQ = HALF // 4              # output store quarter: 8192 px per half

_cached = None


def _build():
    nc = bacc.Bacc("TRN2", target_bir_lowering=False, debug=False)

    x_d = nc.dram_tensor("x", [C, N], BF16, kind="ExternalInput")
    wq_d = nc.dram_tensor("wq", [64, 64], BF16, kind="ExternalInput")
    wkv_d = nc.dram_tensor("wkv", [64, 128], BF16, kind="ExternalInput")
    pj_d = nc.dram_tensor("pj", [64, 64], BF16, kind="ExternalInput")
    y_d = nc.dram_tensor("y", [C, N], BF16, kind="ExternalOutput")

    x_ap = x_d.ap()
    y_ap = y_d.ap()

    with tile.TileContext(nc) as tc:
        with (
            tc.tile_pool(name="persist", bufs=1) as persist,
            tc.tile_pool(name="stash", bufs=1) as stash_pool,
        ):
            wq = persist.tile([64, 64], BF16)
            wkv = persist.tile([64, 128], BF16)
            pj = persist.tile([64, 64], BF16)
            w2 = persist.tile([128, 64], BF16)
            kvbd = persist.tile([64, 64], BF16)
            nc.sync.dma_start(wq[:], wq_d.ap())
            nc.sync.dma_start(wkv[:], wkv_d.ap())
            nc.sync.dma_start(pj[:], pj_d.ap())
            nc.gpsimd.memset(kvbd[:], 0.0)

            # phi(q) stash: c-major, half A rows 0:64, half B rows 64:128
            stash = stash_pool.tile([128, HALF], BF16)
            # y accumulation buffer (bf16), same partition layout as stash
            ybuf = stash_pool.tile([128, HALF], BF16)

            # ---------------- pass 1 ----------------
            with (
                tc.tile_pool(name="xb", bufs=2) as xb_pool,
                tc.tile_pool(name="p1sb", bufs=4) as p1sb,
                tc.tile_pool(name="qps", bufs=2, space="PSUM") as qps_pool,
                tc.tile_pool(name="kvtps", bufs=2, space="PSUM") as kvt_pool,
                tc.tile_pool(name="kvacc", bufs=1, space="PSUM") as kvacc_pool,
            ):
                kvacc = kvacc_pool.tile([64, 64], F32, tag="kvacc")

                xc = None
                for t in range(NTILES):
                    cs = bass.ts(t, NT)  # column slice within each half

                    # big chunked loads: one DMA per CHUNK_PX-pixel chunk
                    # (cols 0:CHUNK_PX half A, CHUNK_PX:2*CHUNK_PX half B)
                    tl = t % TPC
                    if tl == 0:
                        cidx = t // TPC
                        xc = xb_pool.tile([64, 2 * CHUNK_PX], BF16)
                        nc.sync.dma_start(
                            xc[:],
                            bass.AP(
                                x_d, cidx * CHUNK_PX,
                                [[N, 64], [HALF, 2], [1, CHUNK_PX]],
                            ),
                        )
                    xA = xc[:, tl * NT:(tl + 1) * NT]
                    xB = xc[:, CHUNK_PX + tl * NT:CHUNK_PX + (tl + 1) * NT]

                    # q (c-major): halves packed on partitions
                    q_ps = qps_pool.tile([128, NT], F32)
                    nc.tensor.matmul(
                        q_ps[0:64, :], wq[:], xA,
                        start=True, stop=True,
                    )
                    nc.tensor.matmul(
                        q_ps[64:128, :], wq[:], xB,
                        start=True, stop=True, tile_position=(0, 64),
                    )
                    # kT/vT chunks: x-chunk stationary; chunk s cols
                    # [s*128, s*128+64) = kT, [+64, +128) = vT
                    kvt = kvt_pool.tile([128, CHUNKS * 128], F32)
                    for s in range(CHUNKS):
                        if s < CHUNKS // 2:
                            xs = xA[:, bass.ts(s, 128)]
                        else:
                            xs = xB[:, bass.ts(s - CHUNKS // 2, 128)]
                        nc.tensor.matmul(
                            kvt[:, bass.ts(s, 128)],
                            xs,
                            wkv[:],
                            start=True, stop=True,
                        )
                    kvt3 = kvt[:].rearrange("p (s c) -> p s c", s=CHUNKS)

                    # phi = max(x,0) + min(exp(x),1) for q and kT
                    eqk = p1sb.tile([128, 2 * NT], BF16, tag="eqk")
                    nc.scalar.activation(eqk[:, 0:NT], q_ps[:], AF.Exp)
                    nc.scalar.activation(
                        eqk[:, NT:2 * NT].rearrange("p (s c) -> p s c", s=CHUNKS),
                        kvt3[:, :, 0:64], AF.Exp,
                    )
                    mine = p1sb.tile([128, 2 * NT], BF16, tag="mine")
                    nc.vector.tensor_scalar_min(mine[:], eqk[:], 1.0)
                    nc.vector.scalar_tensor_tensor(
                        stash[:, cs], q_ps[:], 0.0, mine[:, 0:NT],
                        op0=ALU.max, op1=ALU.add,
                    )
                    kphiT = p1sb.tile([128, NT], BF16, tag="kphiT")
                    nc.vector.scalar_tensor_tensor(
                        kphiT[:].rearrange("p (s c) -> p s c", s=CHUNKS),
                        kvt3[:, :, 0:64], 0.0,
                        mine[:, NT:2 * NT].rearrange("p (s c) -> p s c", s=CHUNKS),
                        op0=ALU.max, op1=ALU.add,
                    )
                    # vT -> SBUF bf16; split between ACT and DVE for balance
                    vt = p1sb.tile([128, NT], BF16, tag="vt")
                    nc.scalar.copy(
                        vt[:, 0:NT // 2].rearrange("p (s c) -> p s c", s=CHUNKS // 2),
                        kvt3[:, 0:CHUNKS // 2, 64:128],
                    )
                    nc.vector.tensor_copy(
                        vt[:, NT // 2:NT].rearrange("p (s c) -> p s c", s=CHUNKS // 2),
                        kvt3[:, CHUNKS // 2:CHUNKS, 64:128],
                    )

                    # KV^T accumulation (both heads; off-diag ignored later)
                    for s in range(CHUNKS):
                        nc.tensor.matmul(
                            kvacc[:],
                            vt[:, bass.ts(s, 64)],
                            kphiT[:, bass.ts(s, 64)],
                            start=(t == 0 and s == 0),
                            stop=(t == NTILES - 1 and s == CHUNKS - 1),
                            skip_group_check=True,
                        )

                # block-diagonal KV^T (cross-head garbage dropped)
                for h0, h1 in ((0, 32), (32, 64)):
                    nc.vector.tensor_copy(
                        kvbd[h0:h1, h0:h1], kvacc[h0:h1, h0:h1]
                    )

            # ---------------- boundary: W2 = blockdiag(KV) @ proj.T ------
            with tc.tile_pool(name="bps", bufs=1, space="PSUM") as bps:
                w2ps = bps.tile([64, 64], F32)
                nc.tensor.matmul(w2ps[:], kvbd[:], pj[:], start=True, stop=True)
                nc.vector.tensor_copy(w2[0:64, :], w2ps[:])
                nc.vector.tensor_copy(w2[64:128, :], w2ps[:])

            # ---------------- pass 2: y = W2.T @ phi(q) ----------------
            with (
                tc.tile_pool(name="yps", bufs=3, space="PSUM") as yps_pool,
            ):
                tiles_per_q = YQ // NT
                for t in range(NTILES):
                    cs = bass.ts(t, NT)
                    y_ps = yps_pool.tile([128, NT], F32)
                    nc.tensor.matmul(
                        y_ps[0:64, :], w2[0:64, :], stash[0:64, cs],
                        start=True, stop=True,
                    )
                    nc.tensor.matmul(
                        y_ps[64:128, :], w2[64:128, :], stash[64:128, cs],
                        start=True, stop=True, tile_position=(64, 64),
                    )
                    # evacuate PSUM -> ybuf bf16, alternating ACT/DVE
                    if t % 2 == 0:
                        nc.scalar.copy(ybuf[:, cs], y_ps[:])
                    else:
                        nc.vector.tensor_copy(ybuf[:, cs], y_ps[:])
                    # one big store per quarter, overlapped with compute
                    if (t + 1) % tiles_per_q == 0:
                        qt = t // tiles_per_q
                        nc.sync.dma_start(
                            bass.AP(
                                y_d, qt * YQ,
                                [[HALF, 2], [N, 64], [1, YQ]],
                            ),
                            ybuf[:, bass.ts(qt, YQ)],
                        )

    nc.compile()
    return nc


def _get_nc():
    global _cached
    if _cached is None:
        _cached = _build()
    return _cached


def _prep_weights(qkv_w, proj_w):
    wq = np.ascontiguousarray(qkv_w[0:64].T).astype(ml_dtypes.bfloat16)
    wkT = qkv_w[64:128].T
    wvT = qkv_w[128:192].T
    wkv = np.ascontiguousarray(
        np.concatenate([wkT, wvT], axis=1)
    ).astype(ml_dtypes.bfloat16)
    pj = np.ascontiguousarray(proj_w.T).astype(ml_dtypes.bfloat16)
    return wq, wkv, pj


def run(x, qkv_w, proj_w, trace=False):
    nc = _get_nc()
    wq, wkv, pj = _prep_weights(np.asarray(qkv_w), np.asarray(proj_w))
    x = np.asarray(x)
    in_maps = [
        {
            "x": np.ascontiguousarray(x[b].reshape(C, N)).astype(
                ml_dtypes.bfloat16
            ),
            "wq": wq,
            "wkv": wkv,
            "pj": pj,
        }
        for b in range(B)
    ]
    res = run_bass_kernel_spmd(nc, in_maps, core_ids=list(range(B)), trace=trace)
    out = np.stack([res.results[b]["y"].reshape(C, H, W) for b in range(B)])
    return out.astype(np.float32), res


def kernel(x, qkv_w, proj_w):
    out, _ = run(x, qkv_w, proj_w, trace=False)
    return out



# revision 8
# speedup vs baseline: 1.4172x; 1.0013x over previous
"""Linearized attention Trainium2 kernel.

Reference computation per batch b (C=64 channels, H=W=256, N=65536 pixels,
2 heads x 32 head-dim):
    qkv   = qkv_w @ x                      # per-pixel 1x1 conv
    q,k,v = split(qkv); phi(t) = elu(t)+1
    KV    = phi(k) @ v.T  (per head, contract over pixels)   # [32, 32]
    out_h = KV.T @ phi(q) (per head)
    y     = proj_w @ out_h

Sharding: data-parallel over batch, 1 batch per NeuronCore (8 cores).

Kernel structure per core:
- x is fed as bf16 (host cast; device matmuls are bf16 anyway).
- The 65536-pixel image is two half-images ("A" = pixels 0:32768,
  "B" = 32768:65536). SBUF tiles pack A on partitions 0:64, B on 64:128
  where possible, but every matmul keeps its contraction on partitions
  0:64 (mixing disjoint K row-groups between consecutive 128-column
  bf16-weight matmuls crashes the PE array - FWL row-group hazard).
- phi is exact:  phi(x) = max(x, 0) + min(exp(x), 1).
- Pass 1, per [*, 512]-column tile: load x -> one PSUM tile holds
  q (c-major, cols 0:512), kT (pixel-major, cols 512:1024) and vT
  (cols 1024:1536); kT/vT come from matmuls with the x-chunk as the
  *stationary* operand writing a strided 3-D out AP (no transpose
  instructions). One ACT exp covers q+kT; one DVE 4x min clamps; two
  fused (max 0)+add ops produce the phi(q) stash slice and phi(k)T.
  KV^T accumulates in PSUM over all 512 pixel-chunks.
- Boundary: W2 = blockdiag(KV) @ proj_w.T folded into a single [64,64]
  weight, so pass 2 is just  y = W2.T @ phi(q)  per tile + store.
"""

import sys

if "/opt/trn_rl_repo" not in sys.path:
    sys.path.insert(0, "/opt/trn_rl_repo")

import numpy as np
import ml_dtypes

import concourse.bacc as bacc
import concourse.bass as bass
import concourse.mybir as mybir
import concourse.tile as tile
from concourse.bass_utils import run_bass_kernel_spmd

AF = mybir.ActivationFunctionType
ALU = mybir.AluOpType
F32 = mybir.dt.float32
BF16 = mybir.dt.bfloat16

B, C, H, W = 8, 64, 256, 256
N = H * W            # pixels per batch
HALF = N // 2        # pixels per half-image
NT = 512             # tile width (columns per half-image per tile)
NTILES = HALF // NT  # 64
CHUNKS = 2 * NT // 128  # transposed 128-pixel chunks per tile: 8
CHUNK_PX = 4096      # pixels per half per input-load chunk
NCHUNKS = HALF // CHUNK_PX  # 8 input DMAs
TPC = CHUNK_PX // NT        # tiles per load chunk: 8
YQ = HALF // 4              # output store quarter: 8192 px per half

_cached = None


def _build():
    nc = bacc.Bacc("TRN2", target_bir_lowering=False, debug=False)

    x_d = nc.dram_tensor("x", [C, N], BF16, kind="ExternalInput")
    wq_d = nc.dram_tensor("wq", [64, 64], BF16, kind="ExternalInput")
    wkv_d = nc.dram_tensor("wkv", [64, 128], BF16, kind="ExternalInput")
    pj_d = nc.dram_tensor("pj", [64, 64], BF16, kind="ExternalInput")
    y_d = nc.dram_tensor("y", [C, N], BF16, kind="ExternalOutput")

    x_ap = x_d.ap()
    y_ap = y_d.ap()

    with tile.TileContext(nc) as tc:
        with (
            tc.tile_pool(name="persist", bufs=1) as persist,
            tc.tile_pool(name="stash", bufs=1) as stash_pool,
        ):
            wq = persist.tile([64, 64], BF16)
            wkv = persist.tile([64, 128], BF16)
            pj = persist.tile([64, 64], BF16)
            w2 = persist.tile([128, 64], BF16)
            kvbd = persist.tile([64, 64], BF16)
            nc.sync.dma_start(wq[:], wq_d.ap())
            nc.sync.dma_start(wkv[:], wkv_d.ap())
            nc.sync.dma_start(pj[:], pj_d.ap())
            nc.gpsimd.memset(kvbd[:], 0.0)

            # phi(q) stash: c-major, half A rows 0:64, half B rows 64:128
            stash = stash_pool.tile([128, HALF], BF16)
            # y accumulation buffer (bf16), same partition layout as stash
            ybuf = stash_pool.tile([128, HALF], BF16)

            # ---------------- pass 1 ----------------
            with (
                tc.tile_pool(name="xb", bufs=2) as xb_pool,
                tc.tile_pool(name="p1sb", bufs=4) as p1sb,
                tc.tile_pool(name="qps", bufs=2, space="PSUM") as qps_pool,
                tc.tile_pool(name="kvtps", bufs=2, space="PSUM") as kvt_pool,
                tc.tile_pool(name="kvacc", bufs=1, space="PSUM") as kvacc_pool,
            ):
                kvacc = kvacc_pool.tile([64, 64], F32, tag="kvacc")

                xc = None
                for t in range(NTILES):
                    cs = bass.ts(t, NT)  # column slice within each half

                    # big chunked loads: one DMA per CHUNK_PX-pixel chunk
                    # (cols 0:CHUNK_PX half A, CHUNK_PX:2*CHUNK_PX half B)
                    tl = t % TPC
                    if tl == 0:
                        cidx = t // TPC
                        xc = xb_pool.tile([64, 2 * CHUNK_PX], BF16)
                        nc.sync.dma_start(
                            xc[:],
                            bass.AP(
                                x_d, cidx * CHUNK_PX,
                                [[N, 64], [HALF, 2], [1, CHUNK_PX]],
                            ),
                        )
                    xA = xc[:, tl * NT:(tl + 1) * NT]
                    xB = xc[:, CHUNK_PX + tl * NT:CHUNK_PX + (tl + 1) * NT]

                    # q (c-major): halves packed on partitions
                    q_ps = qps_pool.tile([128, NT], F32)
                    nc.tensor.matmul(
                        q_ps[0:64, :], wq[:], xA,
                        start=True, stop=True,
                    )
                    nc.tensor.matmul(
                        q_ps[64:128, :], wq[:], xB,
                        start=True, stop=True, tile_position=(0, 64),
                    )
                    # kT/vT chunks: x-chunk stationary; chunk s cols
                    # [s*128, s*128+64) = kT, [+64, +128) = vT
                    kvt = kvt_pool.tile([128, CHUNKS * 128], F32)
                    for s in range(CHUNKS):
                        if s < CHUNKS // 2:
                            xs = xA[:, bass.ts(s, 128)]
                        else:
                            xs = xB[:, bass.ts(s - CHUNKS // 2, 128)]
                        nc.tensor.matmul(
                            kvt[:, bass.ts(s, 128)],
                            xs,
                            wkv[:],
                            start=True, stop=True,
                        )
                    kvt3 = kvt[:].rearrange("p (s c) -> p s c", s=CHUNKS)

                    # phi = max(x,0) + min(exp(x),1) for q and kT
                    eqk = p1sb.tile([128, 2 * NT], BF16, tag="eqk")
                    nc.scalar.activation(eqk[:, 0:NT], q_ps[:], AF.Exp)
                    nc.scalar.activation(
                        eqk[:, NT:2 * NT].rearrange("p (s c) -> p s c", s=CHUNKS),
                        kvt3[:, :, 0:64], AF.Exp,
                    )
                    mine = p1sb.tile([128, 2 * NT], BF16, tag="mine")
                    nc.vector.tensor_scalar_min(mine[:], eqk[:], 1.0)
                    nc.vector.scalar_tensor_tensor(
                        stash[:, cs], q_ps[:], 0.0, mine[:, 0:NT],
                        op0=ALU.max, op1=ALU.add,
                    )
                    kphiT = p1sb.tile([128, NT], BF16, tag="kphiT")
                    nc.vector.scalar_tensor_tensor(
                        kphiT[:].rearrange("p (s c) -> p s c", s=CHUNKS),
                        kvt3[:, :, 0:64], 0.0,
                        mine[:, NT:2 * NT].rearrange("p (s c) -> p s c", s=CHUNKS),
                        op0=ALU.max, op1=ALU.add,
                    )
                    # vT -> SBUF bf16; split between ACT and DVE for balance
                    vt = p1sb.tile([128, NT], BF16, tag="vt")
                    nc.scalar.copy(
                        vt[:, 0:NT // 2].rearrange("p (s c) -> p s c", s=CHUNKS // 2),
                        kvt3[:, 0:CHUNKS // 2, 64:128],
                    )
                    nc.vector.tensor_copy(
                        vt[:, NT // 2:NT].rearrange("p (s c) -> p s c", s=CHUNKS // 2),
                        kvt3[:, CHUNKS // 2:CHUNKS, 64:128],
                    )

                    # KV^T accumulation (both heads; off-diag ignored later)
                    for s in range(CHUNKS):
                        nc.tensor.matmul(
                            kvacc[:],
                            vt[:, bass.ts(s, 64)],
                            kphiT[:, bass.ts(s, 64)],
                            start=(t == 0 and s == 0),
                            stop=(t == NTILES - 1 and s == CHUNKS - 1),
                            skip_group_check=True,
                        )

                # block-diagonal KV^T (cross-head garbage dropped)
                for h0, h1 in ((0, 32), (32, 64)):
                    nc.vector.tensor_copy(
                        kvbd[h0:h1, h0:h1], kvacc[h0:h1, h0:h1]
                    )

            # ---------------- boundary: W2 = blockdiag(KV) @ proj.T ------
            with tc.tile_pool(name="bps", bufs=1, space="PSUM") as bps:
                w2ps = bps.tile([64, 64], F32)
                nc.tensor.matmul(w2ps[:], kvbd[:], pj[:], start=True, stop=True)
                nc.vector.tensor_copy(w2[0:64, :], w2ps[:])
                nc.vector.tensor_copy(w2[64:128, :], w2ps[:])

            # ---------------- pass 2: y = W2.T @ phi(q) ----------------
            with (
                tc.tile_pool(name="yps", bufs=3, space="PSUM") as yps_pool,
            ):
                tiles_per_q = YQ // NT
                for t in range(NTILES):
                    cs = bass.ts(t, NT)
                    y_ps = yps_pool.tile([128, NT], F32)
                    nc.tensor.matmul(
                        y_ps[0:64, :], w2[0:64, :], stash[0:64, cs],
                        start=True, stop=True,
                    )
                    nc.tensor.matmul(
                        y_ps[64:128, :], w2[64:128, :], stash[64:128, cs],
                        start=True, stop=True, tile_position=(64, 64),
                    )
                    # evacuate PSUM -> ybuf bf16, alternating ACT/DVE
                    if t % 2 == 0:
                        nc.scalar.copy(ybuf[:, cs], y_ps[:])
                    else:
                        nc.vector.tensor_copy(ybuf[:, cs], y_ps[:])
                    # one big store per quarter, overlapped with compute
                    if (t + 1) % tiles_per_q == 0:
                        qt = t // tiles_per_q
                        nc.sync.dma_start(
                            bass.AP(
                                y_d, qt * YQ,
                                [[HALF, 2], [N, 64], [1, YQ]],
                            ),
                            ybuf[:, bass.ts(qt, YQ)],
                        )

    nc.compile()
    return nc


def _get_nc():
    global _cached
    if _cached is None:
        _cached = _build()
    return _cached


def _prep_weights(qkv_w, proj_w):
    wq = np.ascontiguousarray(qkv_w[0:64].T).astype(ml_dtypes.bfloat16)
    wkT = qkv_w[64:128].T
    wvT = qkv_w[128:192].T
    wkv = np.ascontiguousarray(
        np.concatenate([wkT, wvT], axis=1)
    ).astype(ml_dtypes.bfloat16)
    pj = np.ascontiguousarray(proj_w.T).astype(ml_dtypes.bfloat16)
    return wq, wkv, pj


def run(x, qkv_w, proj_w, trace=False):
    nc = _get_nc()
    wq, wkv, pj = _prep_weights(np.asarray(qkv_w), np.asarray(proj_w))
    x = np.asarray(x)
    in_maps = [
        {
            "x": np.ascontiguousarray(x[b].reshape(C, N)).astype(
                ml_dtypes.bfloat16
            ),
            "wq": wq,
            "wkv": wkv,
            "pj": pj,
        }
        for b in range(B)
    ]
    res = run_bass_kernel_spmd(nc, in_maps, core_ids=list(range(B)), trace=trace)
    out = np.stack([res.results[b]["y"].reshape(C, H, W) for b in range(B)])
    return out.astype(np.float32), res


def kernel(x, qkv_w, proj_w):
    out, _ = run(x, qkv_w, proj_w, trace=False)
    return out



# revision 9
# speedup vs baseline: 2.0384x; 1.4383x over previous
"""Linearized attention Trainium2 kernel.

Reference computation per batch b (C=64 channels, H=W=256, N=65536 pixels,
2 heads x 32 head-dim):
    qkv   = qkv_w @ x                      # per-pixel 1x1 conv
    q,k,v = split(qkv); phi(t) = elu(t)+1
    KV    = phi(k) @ v.T  (per head, contract over pixels)   # [32, 32]
    out_h = KV.T @ phi(q) (per head)
    y     = proj_w @ out_h

Sharding: data-parallel over batch, 1 batch per NeuronCore (8 cores).

Kernel structure per core:
- x is fed as bf16 (host cast; device matmuls are bf16 anyway).
- The 65536-pixel image is two half-images ("A" = pixels 0:32768,
  "B" = 32768:65536). SBUF tiles pack A on partitions 0:64, B on 64:128
  where possible, but every matmul keeps its contraction on partitions
  0:64 (mixing disjoint K row-groups between consecutive 128-column
  bf16-weight matmuls crashes the PE array - FWL row-group hazard).
- phi is exact:  phi(x) = max(x, 0) + min(exp(x), 1).
- Pass 1, per [*, 512]-column tile: load x -> one PSUM tile holds
  q (c-major, cols 0:512), kT (pixel-major, cols 512:1024) and vT
  (cols 1024:1536); kT/vT come from matmuls with the x-chunk as the
  *stationary* operand writing a strided 3-D out AP (no transpose
  instructions). One ACT exp covers q+kT; one DVE 4x min clamps; two
  fused (max 0)+add ops produce the phi(q) stash slice and phi(k)T.
  KV^T accumulates in PSUM over all 512 pixel-chunks.
- Boundary: W2 = blockdiag(KV) @ proj_w.T folded into a single [64,64]
  weight, so pass 2 is just  y = W2.T @ phi(q)  per tile + store.
"""

import sys

if "/opt/trn_rl_repo" not in sys.path:
    sys.path.insert(0, "/opt/trn_rl_repo")

import numpy as np
import ml_dtypes

import concourse.bacc as bacc
import concourse.bass as bass
import concourse.mybir as mybir
import concourse.tile as tile
from concourse.bass_utils import run_bass_kernel_spmd

AF = mybir.ActivationFunctionType
ALU = mybir.AluOpType
F32 = mybir.dt.float32
BF16 = mybir.dt.bfloat16

B, C, H, W = 8, 64, 256, 256
N = H * W            # pixels per batch
HALF = N // 2        # pixels per half-image
NT = 512             # tile width (columns per half-image per tile)
NTILES = HALF // NT  # 64
CHUNKS = 2 * NT // 128  # transposed 128-pixel chunks per tile: 8
CHUNK_PX = 4096      # pixels per half per input-load chunk
NCHUNKS = HALF // CHUNK_PX  # 8 input DMAs
TPC = CHUNK_PX // NT        # tiles per load chunk: 8
YQ = HALF // 4              # output store quarter: 8192 px per half

_cached = None


def _build():
    nc = bacc.Bacc("TRN2", target_bir_lowering=False, debug=False)

    x_d = nc.dram_tensor("x", [C, N], BF16, kind="ExternalInput")
    wq_d = nc.dram_tensor("wq", [64, 64], BF16, kind="ExternalInput")
    wkv_d = nc.dram_tensor("wkv", [64, 128], BF16, kind="ExternalInput")
    pj_d = nc.dram_tensor("pj", [64, 64], BF16, kind="ExternalInput")
    y_d = nc.dram_tensor("y", [C, N], BF16, kind="ExternalOutput")

    x_ap = x_d.ap()
    y_ap = y_d.ap()

    with tile.TileContext(nc) as tc:
        with (
            tc.tile_pool(name="persist", bufs=1) as persist,
            tc.tile_pool(name="stash", bufs=1) as stash_pool,
        ):
            wq = persist.tile([64, 64], BF16)
            wkv = persist.tile([64, 128], BF16)
            pj = persist.tile([64, 64], BF16)
            w2 = persist.tile([128, 64], BF16)
            kvbd = persist.tile([64, 64], BF16)
            nc.sync.dma_start(wq[:], wq_d.ap())
            nc.sync.dma_start(wkv[:], wkv_d.ap())
            nc.sync.dma_start(pj[:], pj_d.ap())
            nc.gpsimd.memset(kvbd[:], 0.0)

            # phi(q) stash: c-major, half A rows 0:64, half B rows 64:128
            stash = stash_pool.tile([128, HALF], BF16)
            # y accumulation buffer (bf16), same partition layout as stash
            ybuf = stash_pool.tile([128, HALF], BF16)

            # ---------------- pass 1 ----------------
            with (
                tc.tile_pool(name="xb", bufs=2) as xb_pool,
                tc.tile_pool(name="p1sb", bufs=4) as p1sb,
                tc.tile_pool(name="qps", bufs=2, space="PSUM") as qps_pool,
                tc.tile_pool(name="kvtps", bufs=2, space="PSUM") as kvt_pool,
                tc.tile_pool(name="kvacc", bufs=1, space="PSUM") as kvacc_pool,
            ):
                kvacc = kvacc_pool.tile([64, 64], F32, tag="kvacc")

                xc = None
                for t in range(NTILES):
                    cs = bass.ts(t, NT)  # column slice within each half

                    # big chunked loads: one DMA per CHUNK_PX-pixel chunk
                    # (cols 0:CHUNK_PX half A, CHUNK_PX:2*CHUNK_PX half B)
                    tl = t % TPC
                    if tl == 0:
                        cidx = t // TPC
                        xc = xb_pool.tile([64, 2 * CHUNK_PX], BF16)
                        nc.sync.dma_start(
                            xc[:, 0:CHUNK_PX],
                            bass.AP(
                                x_d, cidx * CHUNK_PX,
                                [[N, 64], [1, CHUNK_PX]],
                            ),
                        )
                        nc.scalar.dma_start(
                            xc[:, CHUNK_PX:2 * CHUNK_PX],
                            bass.AP(
                                x_d, HALF + cidx * CHUNK_PX,
                                [[N, 64], [1, CHUNK_PX]],
                            ),
                        )
                    xA = xc[:, tl * NT:(tl + 1) * NT]
                    xB = xc[:, CHUNK_PX + tl * NT:CHUNK_PX + (tl + 1) * NT]

                    # q (c-major): halves packed on partitions
                    q_ps = qps_pool.tile([128, NT], F32)
                    nc.tensor.matmul(
                        q_ps[0:64, :], wq[:], xA,
                        start=True, stop=True,
                    )
                    nc.tensor.matmul(
                        q_ps[64:128, :], wq[:], xB,
                        start=True, stop=True, tile_position=(0, 64),
                    )
                    # kT/vT chunks: x-chunk stationary; chunk s cols
                    # [s*128, s*128+64) = kT, [+64, +128) = vT
                    kvt = kvt_pool.tile([128, CHUNKS * 128], F32)
                    for s in range(CHUNKS):
                        if s < CHUNKS // 2:
                            xs = xA[:, bass.ts(s, 128)]
                        else:
                            xs = xB[:, bass.ts(s - CHUNKS // 2, 128)]
                        nc.tensor.matmul(
                            kvt[:, bass.ts(s, 128)],
                            xs,
                            wkv[:],
                            start=True, stop=True,
                        )
                    kvt3 = kvt[:].rearrange("p (s c) -> p s c", s=CHUNKS)

                    # phi = max(x,0) + min(exp(x),1) for q and kT
                    eqk = p1sb.tile([128, 2 * NT], BF16, tag="eqk")
                    nc.scalar.activation(eqk[:, 0:NT], q_ps[:], AF.Exp)
                    nc.scalar.activation(
                        eqk[:, NT:2 * NT].rearrange("p (s c) -> p s c", s=CHUNKS),
                        kvt3[:, :, 0:64], AF.Exp,
                    )
                    mine = p1sb.tile([128, 2 * NT], BF16, tag="mine")
                    nc.vector.tensor_scalar_min(mine[:], eqk[:], 1.0)
                    nc.vector.scalar_tensor_tensor(
                        stash[:, cs], q_ps[:], 0.0, mine[:, 0:NT],
                        op0=ALU.max, op1=ALU.add,
                    )
                    kphiT = p1sb.tile([128, NT], BF16, tag="kphiT")
                    nc.vector.scalar_tensor_tensor(
                        kphiT[:].rearrange("p (s c) -> p s c", s=CHUNKS),
                        kvt3[:, :, 0:64], 0.0,
                        mine[:, NT:2 * NT].rearrange("p (s c) -> p s c", s=CHUNKS),
                        op0=ALU.max, op1=ALU.add,
                    )
                    # vT -> SBUF bf16; split between ACT and DVE for balance
                    vt = p1sb.tile([128, NT], BF16, tag="vt")
                    nc.scalar.copy(
                        vt[:, 0:NT // 2].rearrange("p (s c) -> p s c", s=CHUNKS // 2),
                        kvt3[:, 0:CHUNKS // 2, 64:128],
                    )
                    nc.vector.tensor_copy(
                        vt[:, NT // 2:NT].rearrange("p (s c) -> p s c", s=CHUNKS // 2),
                        kvt3[:, CHUNKS // 2:CHUNKS, 64:128],
                    )

                    # KV^T accumulation (both heads; off-diag ignored later)
                    for s in range(CHUNKS):
                        nc.tensor.matmul(
                            kvacc[:],
                            vt[:, bass.ts(s, 64)],
                            kphiT[:, bass.ts(s, 64)],
                            start=(t == 0 and s == 0),
                            stop=(t == NTILES - 1 and s == CHUNKS - 1),
                            skip_group_check=True,
                        )

                # block-diagonal KV^T (cross-head garbage dropped)
                for h0, h1 in ((0, 32), (32, 64)):
                    nc.vector.tensor_copy(
                        kvbd[h0:h1, h0:h1], kvacc[h0:h1, h0:h1]
                    )

            # ---------------- boundary: W2 = blockdiag(KV) @ proj.T ------
            with tc.tile_pool(name="bps", bufs=1, space="PSUM") as bps:
                w2ps = bps.tile([64, 64], F32)
                nc.tensor.matmul(w2ps[:], kvbd[:], pj[:], start=True, stop=True)
                nc.vector.tensor_copy(w2[0:64, :], w2ps[:])
                nc.vector.tensor_copy(w2[64:128, :], w2ps[:])

            # ---------------- pass 2: y = W2.T @ phi(q) ----------------
            with (
                tc.tile_pool(name="yps", bufs=3, space="PSUM") as yps_pool,
            ):
                tiles_per_q = YQ // NT
                for t in range(NTILES):
                    cs = bass.ts(t, NT)
                    y_ps = yps_pool.tile([128, NT], F32)
                    nc.tensor.matmul(
                        y_ps[0:64, :], w2[0:64, :], stash[0:64, cs],
                        start=True, stop=True,
                    )
                    nc.tensor.matmul(
                        y_ps[64:128, :], w2[64:128, :], stash[64:128, cs],
                        start=True, stop=True, tile_position=(64, 64),
                    )
                    # evacuate PSUM -> ybuf bf16, alternating ACT/DVE
                    if t % 2 == 0:
                        nc.scalar.copy(ybuf[:, cs], y_ps[:])
                    else:
                        nc.vector.tensor_copy(ybuf[:, cs], y_ps[:])
                    # one big store per quarter, overlapped with compute
                    if (t + 1) % tiles_per_q == 0:
                        qt = t // tiles_per_q
                        nc.sync.dma_start(
                            bass.AP(
                                y_d, qt * YQ,
                                [[N, 64], [1, YQ]],
                            ),
                            ybuf[0:64, bass.ts(qt, YQ)],
                        )
                        nc.scalar.dma_start(
                            bass.AP(
                                y_d, HALF + qt * YQ,
                                [[N, 64], [1, YQ]],
                            ),
                            ybuf[64:128, bass.ts(qt, YQ)],
                        )

    nc.compile()
    return nc


def _get_nc():
    global _cached
    if _cached is None:
        _cached = _build()
    return _cached


def _prep_weights(qkv_w, proj_w):
    wq = np.ascontiguousarray(qkv_w[0:64].T).astype(ml_dtypes.bfloat16)
    wkT = qkv_w[64:128].T
    wvT = qkv_w[128:192].T
    wkv = np.ascontiguousarray(
        np.concatenate([wkT, wvT], axis=1)
    ).astype(ml_dtypes.bfloat16)
    pj = np.ascontiguousarray(proj_w.T).astype(ml_dtypes.bfloat16)
    return wq, wkv, pj


def run(x, qkv_w, proj_w, trace=False):
    nc = _get_nc()
    wq, wkv, pj = _prep_weights(np.asarray(qkv_w), np.asarray(proj_w))
    x = np.asarray(x)
    in_maps = [
        {
            "x": np.ascontiguousarray(x[b].reshape(C, N)).astype(
                ml_dtypes.bfloat16
            ),
            "wq": wq,
            "wkv": wkv,
            "pj": pj,
        }
        for b in range(B)
    ]
    res = run_bass_kernel_spmd(nc, in_maps, core_ids=list(range(B)), trace=trace)
    out = np.stack([res.results[b]["y"].reshape(C, H, W) for b in range(B)])
    return out.astype(np.float32), res


def kernel(x, qkv_w, proj_w):
    out, _ = run(x, qkv_w, proj_w, trace=False)
    return out

